# revision 2
# baseline (speedup 1.0000x reference)
"""Trainium2 Bass kernel for nn_BoundaryLoss (Sobel-boundary BCE loss), v2.

loss = mean(softplus(z) - z*et) over B=64 images of 512x512, where
  ps  = sigmoid(p)
  z   = |conv(ps,GX)| + |conv(ps,GY)|          (SAME zero padding)
  et  = ((|conv(t,GX)| + |conv(t,GY)|) > 0)    (t binary)

Identity used on device (per pixel):
  softplus(z) - z*et = -ln(sigmoid(z)) + z*(1 - et)
so no per-pixel sign-fold is needed: the et-dependence is the masked sum
m = z*(1-et), accumulated for free by a scalar_tensor_tensor instruction.

Device strategy (8 cores, data parallel over batch):
  * p shipped bf16, t fp8e4m3 (exact).  sigmoid(p) -> fp8 ps (ACT),
    rel-err of fp8 ps on the final loss measured at 6.4e-4 (<< 2e-2).
  * All convs as fp8 DoubleRow banded matmuls (0.5 cyc/row): per
    126-row band unit: ex = conv(ps,GX) in ONE DR stream (GX middle
    column is zero -> stride-2 shift interleave), ey = conv(ps,GY) in 2
    DR streams, wt = conv(t, GX+9*GY) in 2 DR streams (wt != 0 <=> et=1,
    exact in integer arithmetic).
  * z = |ex|+|ey| in ONE pass: tensor_reduce(op=add,
    apply_absolute_value=True) over the (ex,ey) psum pair axis -> bf16.
    Split between DVE and GPSIMD per-unit to balance load.
  * m-sum: scalar_tensor_tensor((wt==0) * z) with accum_out, one instr
    per unit pair, reading the wt psum bank pair directly.
  * sigma(z) on ACT (same table as sigmoid(p), zero table thrash),
    products of 8 units on DVE (bf16, 4x mode), one final Ln-accum pass
    over 1/8 of the data. ln(sigma) in [ -8.xx, ln .5 ] - no underflow.
  * H split into 4 bands of 126 rows + one packed band of the last 8
    rows of all 8 images (72 partitions) -> no halo corrections.
  * Host (float64): adds the w=0 image column, subtracts the phantom
    col-512 contribution and the softplus(0)=ln2 of the structurally
    zero 128 junk partition-rows (both inherited from the band tiling).
"""

import os
import sys

import numpy as np

for _p in ("/opt/trn_rl_repo", os.path.expanduser("~/.axon_site/_ro/trn_rl_repo")):
    if os.path.isdir(_p) and _p not in sys.path:
        sys.path.append(_p)

import concourse.bass as bass
import concourse.bacc as bacc
import concourse.tile as tile
from concourse import mybir
from concourse.bass import _add_dep_helper
from concourse.bass_utils import run_bass_kernel_spmd

F32 = mybir.dt.float32
BF16 = mybir.dt.bfloat16
U32 = mybir.dt.uint32
FP8 = mybir.dt.float8e4
U8 = mybir.dt.uint8
A = mybir.AluOpType
AF = mybir.ActivationFunctionType
AX = mybir.AxisListType
DR = mybir.MatmulPerfMode.DoubleRow

NCORES = 8
B, H, W = 64, 512, 512
BPC = B // NCORES          # images per core
NB = 4                     # full 126-row bands per image
BAND = 126
NBP = BPC * NB + 1         # band units per core (33)
NPAIR = (NBP + 1) // 2     # wt pair instrs (17)
PADW = 4                   # zero pad cols for shifts 0..3
LNSTRIDE = 8               # units multiplied together per Ln element

# 3x3 kernels
_GX = np.array([[1., 0., -1.], [2., 0., -2.], [1., 0., -1.]])
_GY = np.array([[1., 2., 1.], [0., 0., 0.], [-1., -2., -1.]])
_WK = _GX + 9.0 * _GY
_Z3 = np.zeros(3)

# streams: (taps, shift_base, shift_stride, branch, start, stop)
# branch 0 = ps psum half ex, 1 = ps psum half ey, 2 = wt.
# A DoubleRow matmul must OPEN its psum accumulation group (DR with
# start=False wedges the exec unit), so 3-tap convs are DR + plain fp8.
_STREAMS = [
    ([_GX[:, 0], _GX[:, 2]], 0, 2, 0, True, True),   # ex: cols 0,2 (DR)
    ([_GY[:, 0], _GY[:, 1]], 0, 1, 1, True, False),  # ey: cols 0,1 (DR)
    ([_GY[:, 2]], 2, 1, 1, False, True),             # ey: col 2 (plain)
    ([_WK[:, 0], _WK[:, 1]], 0, 1, 2, True, False),  # wt: cols 0,1 (DR)
    ([_WK[:, 2]], 2, 1, 2, False, True),             # wt: col 2 (plain)
]
NSTR = len(_STREAMS)
# lhsT slot offset of each stream within a variant block (pairs take 2)
_SLOT = []
_off = 0
for _s in _STREAMS:
    _SLOT.append(_off)
    _off += len(_s[0])
VSLOTS = _off          # lhsT matrices per variant (8)

# per-unit z engine: 'd' = DVE reduce; 'a' = ACT abs-pair + Pool add
# (GPSIMD cannot touch PSUM; ACT assists move drain work off DVE).
# Assists sit at the END of each 8-unit sigma chunk so the ACT queue
# reaches them right before the sigma that consumes their output.
Z_ENG = ['a' if u % 4 == 2 else 'd' for u in range(NBP)]
# per-pair m engine: 'd' = DVE stt ('a' is rejected by the backend)
M_ENG = ['d'] * NPAIR
# product-tree level-1 engine ('p' = Pool frees DVE)
PRODS_L1 = 'd'
# sigma(z) ACT instruction granularity in units (product groups stay 8)
SIG_CHUNK = 4
# emit sigmoid(p) per image instead of per pair
SIGP_SPLIT = False
# number of PE warmup dummy matmuls (0 = off)
PE_WARM = 0


def _banded(tap, variant):
    """[128, 128] f32 lhsT for a vertical 3-tap `tap` under `variant`:
    0 = interior (T[p,q] = tap[p-q]), 1 = band0 (T[p,q] = tap[p-q+1]),
    2 = packed block-diag 9->8 rows per image."""
    m = np.zeros((128, 128), np.float32)
    if variant == 2:
        for j in range(BPC):
            for qq in range(8):
                for dh in range(3):
                    pp = qq + dh
                    if pp < 9:
                        m[9 * j + pp, 8 * j + qq] = tap[dh]
        return m
    off = 1 if variant == 1 else 0
    for q in range(BAND):
        for dh in range(3):
            p = q + dh - off
            if 0 <= p < 128:
                m[p, q] = tap[dh]
    return m


def _lhst_mats():
    """[128, 3*VSLOTS, 128] fp8: variant-major, DR pairs adjacent."""
    out = np.zeros((3 * VSLOTS, 128, 128), np.float32)
    for v in range(3):
        for s, (taps, _, _, _, _, _) in enumerate(_STREAMS):
            for j, tap in enumerate(taps):
                out[v * VSLOTS + _SLOT[s] + j] = _banded(tap, v)
    return np.ascontiguousarray(out.transpose(1, 0, 2)).astype(
        mybir.dt.np(FP8))


def _build_program(opts=()):
    opts = set(opts)
    nc = bacc.Bacc("TRN2", target_bir_lowering=False)
    p_d = nc.dram_tensor("p", [BPC, H, W], BF16, kind="ExternalInput")
    t_d = nc.dram_tensor("t", [BPC, H, W], FP8, kind="ExternalInput")
    bw_d = nc.dram_tensor("bw", [128, 3 * VSLOTS, 128], FP8,
                          kind="ExternalInput")
    out_d = nc.dram_tensor("out", [128, NPAIR + 1], F32,
                           kind="ExternalOutput")
    if "debug" in opts:
        dbg_z = nc.dram_tensor("dbg_z", [128, NBP, W], BF16,
                               kind="ExternalOutput")

    WP = W + PADW

    def band_aps(dram, i0, n, wbytes):
        """(b0 ap, b123 ap) for images i0..i0+n-1 of dram [BPC,H,W]."""
        t = dram[:, :, :].tensor
        b0 = bass.AP(tensor=t, offset=i0 * H * W,
                     ap=[[W, 128], [H * W, n], [1, W]])
        b123 = bass.AP(tensor=t, offset=i0 * H * W + (BAND - 1) * W,
                       ap=[[W, 128], [H * W, n], [BAND * W, 3], [1, W]])
        return b0, b123

    with tile.TileContext(nc) as tc:
        with tc.tile_pool(name="consts", bufs=1) as consts, \
             tc.tile_pool(name="xin", bufs=1) as xin, \
             tc.tile_pool(name="tin", bufs=1) as tin, \
             tc.tile_pool(name="psg", bufs=1) as psg, \
             tc.tile_pool(name="packed", bufs=1) as packed, \
             tc.tile_pool(name="zs", bufs=1) as zs, \
             tc.tile_pool(name="scrap", bufs=1) as scrap, \
             tc.tile_pool(name="accp", bufs=1) as accp, \
             tc.tile_pool(name="psum", bufs=2, space="PSUM") as psum, \
             tc.tile_pool(name="psuma", bufs=1, space="PSUM") as psuma, \
             tc.tile_pool(name="psum2", bufs=1, space="PSUM") as psum2:

            bw = consts.tile([128, 3 * VSLOTS, 128], FP8)

            zq = zs.tile([128, NBP, W], BF16)
            sq = zs.tile([128, NBP, W], BF16)
            mscrap_d = scrap.tile([128, 2, W], BF16)
            mscrap_p = scrap.tile([128, 2, W], BF16)
            acc = accp.tile([128, NPAIR + 1], F32)

            # ACT warm-up: load the sigmoid table before any real work
            warm = accp.tile([1, 1], F32)
            nc.gpsimd.memset(warm[:, :], 0.0)
            nc.scalar.activation(out=warm[:, :], in_=warm[:, :],
                                 func=AF.Sigmoid)

            units = []      # (variant, ps_view, t_view, kk)
            sig_acts = []
            first = True

            # packed band first: its DMAs read DRAM directly (tiny), so it
            # starts the drain ladder early and stays out of the tail
            p4 = packed.tile([72, W], BF16, tag="p4")
            t4 = packed.tile([72, WP], FP8, tag="t4")
            ps4 = packed.tile([72, WP], FP8, tag="ps4")
            nc.gpsimd.memset(t4[:, W:WP].bitcast(U8), 0)
            nc.gpsimd.memset(ps4[:, W:WP].bitcast(U8), 0)
            src_off = (H - 9) * W
            nc.sync.dma_start(
                out=p4[0:72, :],
                in_=bass.AP(tensor=p_d[:, :, :].tensor, offset=src_off,
                            ap=[[H * W, 8], [W, 9], [1, W]]))
            nflat = 3 * VSLOTS * 128
            v2off = 2 * VSLOTS * 128
            nc.sync.dma_start(
                out=bass.AP(tensor=bw.tensor, offset=bw.offset + v2off,
                            ap=[[bw.ap[0][0], 128], [1, VSLOTS * 128]]),
                in_=bass.AP(tensor=bw_d[:, :, :].tensor, offset=v2off,
                            ap=[[nflat, 128], [1, VSLOTS * 128]]))
            nc.sync.dma_start(
                out=t4[0:72, 0:W],
                in_=bass.AP(tensor=t_d[:, :, :].tensor, offset=src_off,
                            ap=[[H * W, 8], [W, 9], [1, W]]))
            nc.sync.dma_start(
                out=bass.AP(tensor=bw.tensor, offset=bw.offset,
                            ap=[[bw.ap[0][0], 128], [1, v2off]]),
                in_=bass.AP(tensor=bw_d[:, :, :].tensor, offset=0,
                            ap=[[nflat, 128], [1, v2off]]))
            sa = nc.scalar.activation(out=ps4[:, 0:W], in_=p4[:, :],
                                      func=AF.Sigmoid)
            sig_acts.append(sa)
            units.append((2, ps4[:, :], t4[:, :], 72))

            if PE_WARM:
                # p-state warmup: keep PE continuously busy with dummy
                # matmuls (into a psum buf that unit 1 will overwrite) so
                # the real streams start at full clock
                wp1 = psum.tile([128, 2, W], F32, tag="p1")
                for _ in range(PE_WARM):
                    nc.tensor.matmul(wp1[:, 0, :], p4[0:1, 0:128],
                                     p4[0:1, 0:W], start=True, stop=True)
            for ip in range(BPC // 2):      # image pairs
                x2 = xin.tile([128, 8, W], BF16, tag=f"x{ip}")
                t2 = tin.tile([128, 8, WP], FP8, tag=f"t{ip}")
                ps2 = psg.tile([128, 8, WP], FP8, tag=f"ps{ip}")
                nc.gpsimd.memset(t2[:, :, W:WP].bitcast(U8), 0)
                nc.gpsimd.memset(ps2[:, :, W:WP].bitcast(U8), 0)
                def p_b123(i2):
                    src = bass.AP(
                        tensor=p_d[:, :, :].tensor,
                        offset=(2 * ip + i2) * H * W + (BAND - 1) * W,
                        ap=[[W, 128], [BAND * W, 3], [1, W]])
                    nc.sync.dma_start(
                        out=x2[:, 4 * i2 + 1:4 * i2 + 4, :], in_=src)

                def t_b123(i2):
                    src = bass.AP(
                        tensor=t_d[:, :, :].tensor,
                        offset=(2 * ip + i2) * H * W + (BAND - 1) * W,
                        ap=[[W, 128], [BAND * W, 3], [1, W]])
                    nc.sync.dma_start(
                        out=t2[:, 4 * i2 + 1:4 * i2 + 4, 0:W], in_=src)

                pb0, _ = band_aps(p_d, 2 * ip, 2, 2)
                nc.sync.dma_start(
                    out=bass.AP(tensor=x2.tensor, offset=x2.offset,
                                ap=[[x2.ap[0][0], 128], [4 * W, 2], [1, W]]),
                    in_=pb0)
                p_b123(0)
                tb0, _ = band_aps(t_d, 2 * ip, 2, 1)
                nc.sync.dma_start(
                    out=bass.AP(tensor=t2.tensor, offset=t2.offset,
                                ap=[[t2.ap[0][0], 128], [4 * WP, 2], [1, W]]),
                    in_=tb0)
                t_b123(0)
                if first:
                    # image 0 sigmoid as soon as its bands land
                    sa = nc.scalar.activation(out=ps2[:, 0:4, 0:W],
                                              in_=x2[:, 0:4, :],
                                              func=AF.Sigmoid)
                    sig_acts.append(sa)
                p_b123(1)
                t_b123(1)
                if first:
                    sa = nc.scalar.activation(out=ps2[:, 4:8, 0:W],
                                              in_=x2[:, 4:8, :],
                                              func=AF.Sigmoid)
                    first = False
                elif SIGP_SPLIT:
                    sa = nc.scalar.activation(out=ps2[:, 0:4, 0:W],
                                              in_=x2[:, 0:4, :],
                                              func=AF.Sigmoid)
                    sig_acts.append(sa)
                    sa = nc.scalar.activation(out=ps2[:, 4:8, 0:W],
                                              in_=x2[:, 4:8, :],
                                              func=AF.Sigmoid)
                else:
                    sa = nc.scalar.activation(out=ps2[:, :, 0:W],
                                              in_=x2[:, :, :],
                                              func=AF.Sigmoid)
                sig_acts.append(sa)
                for i2 in range(2):
                    for b in range(NB):
                        units.append((1 if b == 0 else 0,
                                      ps2[:, 4 * i2 + b, :],
                                      t2[:, 4 * i2 + b, :], 128))

            # ---- per-unit compute ----
            def shifted_rhs(view, kk, base, stride, ntap):
                if ntap == 2:
                    return bass.AP(tensor=view.tensor,
                                   offset=view.offset + base,
                                   ap=[[view.ap[0][0], kk], [stride, 2],
                                       [1, W]])
                return bass.AP(tensor=view.tensor, offset=view.offset + base,
                               ap=[[view.ap[0][0], kk], [1, W]])

            wt_pair = None
            sig2_acts = []
            for u, (v, psv, tv, kk) in enumerate(units):
                if Z_ENG[u] == 'a':
                    p1 = psuma.tile([128, 2, W], F32, tag="p1a")
                else:
                    p1 = psum.tile([128, 2, W], F32, tag="p1")
                half = u % 2
                if half == 0:
                    wt_pair = psum2.tile([128, 2, W], F32, tag="p2")
                for s, (taps, base, stride, br, st, sp) in enumerate(_STREAMS):
                    if "no_mm" in opts:
                        break
                    ntap = len(taps)
                    slot = v * VSLOTS + _SLOT[s]
                    lt = (bw[0:kk, slot:slot + 2, :] if ntap == 2
                          else bw[0:kk, slot, :])
                    view = tv if br == 2 else psv
                    dst = wt_pair[:, half, :] if br == 2 else p1[:, br, :]
                    rhs = shifted_rhs(view, kk, base, stride, ntap)
                    nc.tensor.matmul(dst, lt, rhs, start=st, stop=sp,
                                     perf_mode=DR if ntap == 2 else None)
                # z = |ex| + |ey| in one reduce over the pair axis
                if "no_z" in opts:
                    if u == 0:
                        nc.gpsimd.memset(zq[:, 0, :].bitcast(U8), 0)
                elif Z_ENG[u] == 'd':
                    with nc.allow_low_precision(reason="2-term abs-add bf16"):
                        nc.vector.tensor_reduce(
                            out=zq[:, u, :],
                            in_=p1[:, :, :].rearrange("p c w -> p w c"),
                            axis=AX.X, op=A.add, apply_absolute_value=True)
                else:
                    # ACT drains |ex|,|ey| to SBUF bf16; Pool adds them
                    axy = scrap.tile([128, 2, W], BF16, tag="axy")
                    nc.scalar.activation(
                        out=axy[:, :, :].rearrange("p c w -> p (c w)"),
                        in_=p1[:, :, :].rearrange("p c w -> p (c w)"),
                        func=AF.Abs)
                    nc.gpsimd.tensor_tensor(
                        out=zq[:, u, :], in0=axy[:, 0, :], in1=axy[:, 1, :],
                        op=A.add)
                if (half == 1 or u == NBP - 1) and "no_m" not in opts:
                    q = u // 2
                    nhalf = half + 1
                    if M_ENG[q] == 'd':
                        nc.vector.scalar_tensor_tensor(
                            out=mscrap_d[:, 0:nhalf, :],
                            in0=wt_pair[:, 0:nhalf, :],
                            scalar=0.0,
                            in1=zq[:, u - nhalf + 1:u + 1, :],
                            op0=A.is_equal, op1=A.mult,
                            accum_out=acc[:, q:q + 1])
                    else:
                        # ACT drains wt to SBUF bf16 (exact for ints <= 36),
                        # Pool does the mask-mult-accumulate in SBUF
                        w8 = scrap.tile([128, 2, W], BF16, tag="w8")
                        nc.scalar.activation(
                            out=w8[:, 0:nhalf, :].rearrange("p c w -> p (c w)"),
                            in_=wt_pair[:, 0:nhalf, :].rearrange("p c w -> p (c w)"),
                            func=AF.Identity)
                        nc.gpsimd.scalar_tensor_tensor(
                            out=mscrap_p[:, 0:nhalf, :],
                            in0=w8[:, 0:nhalf, :],
                            scalar=0.0,
                            in1=zq[:, u - nhalf + 1:u + 1, :],
                            op0=A.is_equal, op1=A.mult,
                            accum_out=acc[:, q:q + 1])
                # sigma(z) in chunks of SIG_CHUNK units
                if (((u + 1) % SIG_CHUNK == 0 or u == NBP - 1)
                        and "no_tail" not in opts):
                    lo = (u // SIG_CHUNK) * SIG_CHUNK
                    sa = nc.scalar.activation(
                        out=sq[:, lo:u + 1, :].rearrange("p c w -> p (c w)"),
                        in_=zq[:, lo:u + 1, :].rearrange("p c w -> p (c w)"),
                        func=AF.Sigmoid)
                    sig2_acts.append(sa)
                # product tree per group of 8 (bf16, all-SBUF)
                if ((u + 1) % 8 == 0 or u == NBP - 1) and "no_tail" not in opts:
                    lo = (u // 8) * 8
                    n = u + 1 - lo
                    step = 1
                    while step < n:
                        if n // (2 * step) > 0:
                            eng = (nc.gpsimd if (step == 1 and
                                                 PRODS_L1 == 'p')
                                   else nc.vector)
                            eng.tensor_tensor(
                                out=sq[:, lo:lo + n - step:2 * step, :],
                                in0=sq[:, lo:lo + n - step:2 * step, :],
                                in1=sq[:, lo + step:lo + n:2 * step, :],
                                op=A.mult)
                        step *= 2
                    if LNSTRIDE == 16 and (u + 1) % 16 == 0:
                        g = u + 1 - 16
                        nc.vector.tensor_tensor(
                            out=sq[:, g, :], in0=sq[:, g, :],
                            in1=sq[:, g + 8, :], op=A.mult)

            if "debug" in opts:
                nc.sync.dma_start(out=dbg_z[:, :, :], in_=zq[:, :, :])

            # ---- final Ln over the chunk products (slots 0,8,16,24,32) ----
            if "no_tail" not in opts:
                # early m-slot output while the Ln tail runs
                nc.sync.dma_start(out=out_d[:, 0:NPAIR], in_=acc[:, 0:NPAIR])
                # dummy Ln right after the last sigma hoists the table load
                # off the product-tree critical path
                warm1 = accp.tile([1, 1], F32)
                nc.gpsimd.memset(warm1[:, :], 1.0)
                dl = nc.scalar.activation(out=warm1[:, :], in_=warm1[:, :],
                                          func=AF.Ln)
                _add_dep_helper(dl.ins, sig2_acts[-1].ins, sync=True,
                                reason="ACT table phase split")
                li = nc.scalar.activation(
                    out=zq[:, 0:NBP:LNSTRIDE, :],
                    in_=sq[:, 0:NBP:LNSTRIDE, :],
                    func=AF.Ln, accum_out=acc[:, NPAIR:NPAIR + 1])
                nc.sync.dma_start(out=out_d[:, NPAIR:NPAIR + 1],
                                  in_=acc[:, NPAIR:NPAIR + 1])
            else:
                nc.vector.tensor_reduce(
                    out=acc[:, NPAIR:NPAIR + 1],
                    in_=zq[:, 0, :], axis=AX.X, op=A.max)
            if "no_m" in opts:
                nc.gpsimd.memset(acc[:, 0:NPAIR], 0.0)
            if "no_tail" in opts:
                nc.sync.dma_start(out=out_d[:, :], in_=acc)

    nc.compile()
    return nc


_NC_CACHE = None


def _get_nc():
    global _NC_CACHE
    if _NC_CACHE is None:
        _NC_CACHE = _build_program()
    return _NC_CACHE


def _edge_loss_sum(p, t):
    """float64 loss sum over the w=0 image column (not computed on device)."""
    ps = 1.0 / (1.0 + np.exp(-p.astype(np.float64)))
    td = t.astype(np.float64)

    def slab(x):
        s = np.zeros((B, H + 2, 3))
        s[:, 1:H + 1, 1:3] = x[:, :, 0:2]
        return s

    sp, st = slab(ps), slab(td)

    def conv(x, K):
        acc = np.zeros((B, H))
        for dh in range(3):
            for dw in range(3):
                acc += K[dh, dw] * x[:, dh:dh + H, dw]
        return acc

    z = np.abs(conv(sp, _GX)) + np.abs(conv(sp, _GY))
    et = (np.abs(conv(st, _GX)) + np.abs(conv(st, _GY))) > 0
    return (np.logaddexp(0.0, z) - z * et).sum()


def _phantom_loss_sum(p, t):
    """float64 loss sum the device adds for its phantom column (image col
    512, fed by image col 511 + zero pads); subtracted from the total."""
    ps_col = 1.0 / (1.0 + np.exp(-p[:, :, W - 1].astype(np.float64)))
    t_col = t[:, :, W - 1].astype(np.float64)

    def vconv(col, tap):
        s = np.zeros((B, H + 2))
        s[:, 1:H + 1] = col
        return tap[0] * s[:, 0:H] + tap[1] * s[:, 1:H + 1] + tap[2] * s[:, 2:H + 2]

    ex = vconv(ps_col, np.array([1.0, 2.0, 1.0]))
    ey = vconv(ps_col, np.array([1.0, 0.0, -1.0]))
    wt = vconv(t_col, np.array([10.0, 2.0, -8.0]))
    z = np.abs(ex) + np.abs(ey)
    et = wt != 0
    return (np.logaddexp(0.0, z) - z * et).sum()


def kernel(p: np.ndarray, t: np.ndarray) -> np.ndarray:
    p = np.ascontiguousarray(np.asarray(p, dtype=np.float32)).reshape(B, H, W)
    t = np.ascontiguousarray(np.asarray(t, dtype=np.float32)).reshape(B, H, W)
    nc = _get_nc()
    bw = _lhst_mats()
    p16 = p.astype(mybir.dt.np(BF16))
    t8 = t.astype(mybir.dt.np(FP8))
    in_maps = [
        {"p": p16[c * BPC:(c + 1) * BPC], "t": t8[c * BPC:(c + 1) * BPC],
         "bw": bw}
        for c in range(NCORES)
    ]
    res = run_bass_kernel_spmd(nc, in_maps, core_ids=list(range(NCORES)))
    # 128 structurally-zero junk partition-rows per core contribute
    # -ln(sigmoid(0)) = ln2 each at W columns through the Ln path.
    junk = (2 * BPC * NB + (128 - 8 * BPC)) * W * np.log(2.0)
    total = 0.0
    for c in range(NCORES):
        o = res.results[c]["out"].astype(np.float64)
        total += o[:, 0:NPAIR].sum() - o[:, NPAIR].sum() - junk
    total += _edge_loss_sum(p, t) - _phantom_loss_sum(p, t)
    return np.float32(total / (B * H * W))
